# revision 7
# baseline (speedup 1.0000x reference)
"""DBSCAN neighbor-count kernel for Trainium2 (8 NeuronCores).

Problem: point_features [4, 8192, 16] f32 -> labels [4, 8192] int32
  d2[b,i,j] = ||x_i - x_j||^2 ; neighbor iff dist < 0.5 (d2 < 0.25)
  label = -1 if neighbor_count < 10 else 0

Strategy (quadrant-symmetric, v3):
  - 8 cores: core c -> batch b=c//2, role r=c%2. Quadrants of the 8192^2
    pair matrix (4096^2 each): Q11, Q12, Q21, Q22. Q21 = Q12^T, so only
    Q11, Q22, Q12 are computed (25% less work):
      role 0: Q11 (rows/cols upper) + left half of Q12 (upper rows x cols 4096:6144)
      role 1: Q22 (rows/cols lower) + right half of Q12 (upper rows x cols 6144:8192)
  - Threshold folded into an augmented Gram matmul (K=18, zero-padded to
    K=128 so every matmul uses the full-array (128,128) tiling mode):
      Gt[i,j] = dot(x_i,x_j) - a_i - b_j > 0  <=>  neighbor
  - Epilogue, one fused pass per [128,1024] PSUM group, split across both
    PSUM-capable engines:
      ScalarE: activation(Sigmoid, scale=1e6, accum_out) -> row-count partial
      VectorE: tensor_scalar(is_gt 0, add, accum_out)    -> row-count partial
    For Q12 groups the mandatory elementwise output IS the 0/1 mask (bf16,
    SBUF), which TensorE then column-sums via an all-ones stationary matrix
    into a persistent PSUM accumulator -> counts for the lower points.
  - Host merges: upper counts = Q11 rowsums + Q12 rowsums (both cores);
    lower counts = Q22 rowsums + Q12 colsums; label = -1 iff count < 9.5.

Dispatch: the bass module is AOT-lowered and compiled ONCE per process via
jax/PJRT (fast_dispatch_compile); subsequent kernel() calls reuse the
compiled executable — no per-call re-trace/re-lower/re-serialize. Outputs
are fully written by the kernel, so no donated zero buffers are needed.
"""
import numpy as np
import ml_dtypes

import jax
from jax.experimental.shard_map import shard_map
from jax.sharding import Mesh, PartitionSpec

import bass_rust
import concourse.bass as bass
import concourse.mybir as mybir
import concourse.tile as tile
from concourse.bass2jax import (
    _bass_exec_p,
    fast_dispatch_compile,
    install_neuronx_cc_hook,
    partition_id_tensor,
)

B, N, D = 4, 8192, 16
HALF = N // 2              # 4096 rows per core
KAUG = 18                  # features + threshold-fold rows
NBLK = HALF // 128         # 32 i-blocks per core
GRP = 1024                 # psum group width (2 banks)
QCOL = 2048                # q12 columns per core
NG_A = HALF // GRP         # 4 diag groups per i-block
NG_B = QCOL // GRP         # 2 q12 groups per i-block
ACT_SHARE = 0.4948         # ScalarE fraction of epilogue groups
N_CORES = 8

_cache = {}


_FOLD_OK = {
    "InstMatmult", "InstLdweights", "InstActivation", "InstTensorScalarPtr",
    "InstTensorReduce", "InstTensorCopy", "InstMemset", "InstTensorTensor",
}


def split_excess_waits(nc, limit=1):
    """This walrus build caps sync-waits per instruction. Move extras onto the
    immediately-preceding same-engine instruction when it carries no wait
    (earlier wait = semantically stronger, same stall point); otherwise hoist
    into standalone NoOps on the same engine."""
    n_split = n_fold = 0
    for f in nc.m.functions:
        for b in f.blocks:
            out = []
            changed = False
            last_by_eng = {}
            for i in b.instructions:
                si = i.sync_info
                if si and si.on_wait and len(si.on_wait) > limit:
                    waits = list(si.on_wait)
                    extra, keep = waits[:-limit], waits[-limit:]
                    rest = []
                    for w in extra:
                        prev = last_by_eng.get(i.engine)
                        psi = prev.sync_info if prev is not None else None
                        if (prev is not None
                                and type(prev).__name__ in _FOLD_OK
                                and (psi is None or not psi.on_wait)):
                            upd = list(psi.on_update) if psi and psi.on_update else []
                            prev.sync_info = bass_rust.SyncInfo(
                                on_wait=[w], on_update=upd)
                            n_fold += 1
                        else:
                            rest.append(w)
                    for k, w in enumerate(rest):
                        nop = mybir.InstNoOp(name=f"{i.name}_xw{k}")
                        nop.engine = i.engine
                        nop.sync_info = bass_rust.SyncInfo(on_wait=[w], on_update=[])
                        out.append(nop)
                        last_by_eng[i.engine] = nop
                        n_split += 1
                    si.on_wait = keep
                    i.sync_info = si
                    changed = True
                out.append(i)
                last_by_eng[i.engine] = i
            if changed:
                b.instructions = out
    return n_split, n_fold


def _build(repeat=1):
    bf16 = mybir.dt.bfloat16
    f32 = mybir.dt.float32
    SIG = mybir.ActivationFunctionType.Sigmoid

    nc = bass.Bass()
    ls_d = nc.dram_tensor("lhs_self", [KAUG, HALF], bf16, kind="ExternalInput")
    lu_d = nc.dram_tensor("lhs_up", [KAUG, HALF], bf16, kind="ExternalInput")
    rd_d = nc.dram_tensor("rhs_diag", [KAUG, HALF], bf16, kind="ExternalInput")
    rq_d = nc.dram_tensor("rhs_q12", [KAUG, QCOL], bf16, kind="ExternalInput")
    cdiag_d = nc.dram_tensor("cdiag", [128, NBLK], f32, kind="ExternalOutput")
    cq12_d = nc.dram_tensor("cq12", [128, NBLK], f32, kind="ExternalOutput")
    colsum_d = nc.dram_tensor("colsum", [1, QCOL], f32, kind="ExternalOutput")

    n_groups = NBLK * (NG_A + NG_B)

    def mk_assign():
        assign = []
        acc = 0.0
        for _ in range(n_groups):
            acc += ACT_SHARE
            if acc >= 1.0:
                assign.append("A")
                acc -= 1.0
            else:
                assign.append("D")
        return assign

    with tile.TileContext(nc) as tc:
        with (
            tc.tile_pool(name="inp", bufs=1) as inp,
            tc.tile_pool(name="psum", bufs=3, space="PSUM") as psum,
            tc.tile_pool(name="pscol", bufs=2, space="PSUM") as pscol,
            tc.tile_pool(name="masks", bufs=6) as maskp,
            tc.tile_pool(name="fin", bufs=1) as fin,
        ):
            ls = inp.tile([128, HALF], bf16, name="ls")
            lu = inp.tile([128, HALF], bf16, name="lu")
            rd = inp.tile([128, HALF], bf16, name="rd")
            rq = inp.tile([128, QCOL], bf16, name="rq")
            ones = fin.tile([128, 128], bf16, name="ones")
            # zero the K-padding rows (18..127); garbage there would
            # NaN-poison. Spread across engines so startup isn't
            # serialized on one. Done once; per-rep DMAs only touch
            # rows 0..KAUG-1.
            nc.gpsimd.memset(ls[:, :].bitcast(mybir.dt.uint32), 0)
            nc.scalar.memzero(rd[:, :])
            nc.gpsimd.memset(lu[:, :].bitcast(mybir.dt.uint32), 0)
            nc.gpsimd.memset(rq[:, :].bitcast(mybir.dt.uint32), 0)
            nc.gpsimd.memset(ones, 1.0)

            parts = fin.tile([128, NBLK * (NG_A + NG_B)], f32, name="parts")
            cdiag = fin.tile([128, NBLK], f32, name="cdiag")
            cq12 = fin.tile([128, NBLK], f32, name="cq12")
            colsum_sb = fin.tile([1, QCOL], f32, name="colsum")

            def body(rep):
                it_assign = iter(mk_assign())
                sfx = f"_r{rep}"
                nc.sync.dma_start(out=ls[0:KAUG, :], in_=ls_d[:, :])
                nc.sync.dma_start(out=lu[0:KAUG, :], in_=lu_d[:, :])
                for jc in range(NG_A):
                    nc.sync.dma_start(out=rd[0:KAUG, jc * GRP:(jc + 1) * GRP],
                                      in_=rd_d[:, jc * GRP:(jc + 1) * GRP])
                nc.sync.dma_start(out=rq[0:KAUG, :], in_=rq_d[:, :])

                def epilogue(pt, slot, eng, mask=None):
                    # phase A (mask=None): write elementwise result in-place
                    # into the PSUM tile — avoids SBUF trash tiles and their
                    # WAW waits.
                    out_t = mask if mask is not None else pt
                    if eng == "A":
                        nc.scalar.activation(out_t, pt, SIG, bias=0.0,
                                             scale=1.0e6, accum_out=slot)
                    else:
                        nc.vector.tensor_scalar(out_t, pt, 0.0, None,
                                                mybir.AluOpType.is_gt,
                                                mybir.AluOpType.add,
                                                accum_out=slot)

                # ---- phase A: diag quadrant (rowsums only) ----
                for t in range(NBLK):
                    for g in range(NG_A):
                        pt = psum.tile([128, GRP], f32, tag="psum",
                                       name=f"psA{t}_{g}{sfx}")
                        for u in range(GRP // 512):
                            c0 = g * GRP + u * 512
                            nc.tensor.matmul(
                                out=pt[:, u * 512:(u + 1) * 512],
                                lhsT=ls[:, t * 128:(t + 1) * 128],
                                rhs=rd[:, c0:c0 + 512],
                                start=True, stop=True)
                        epilogue(pt, parts[:, t * 6 + g: t * 6 + g + 1],
                                 next(it_assign))

                # ---- phase B: q12 strips (rowsums + masks -> PE colsums) ----
                # colsum matmuls for iteration t are emitted during iteration
                # t+1 so PE prefills the next group instead of stalling on
                # epilogue(t).
                for sg in range(NG_B):
                    pc = [pscol.tile([128, 512], f32, tag="pscol",
                                     name=f"pc{sg}_{h}{sfx}")
                          for h in range(2)]

                    def colsums(t, mask):
                        for h in range(2):
                            nc.tensor.matmul(
                                out=pc[h][:, :],
                                lhsT=ones[:, :],
                                rhs=mask[:, h * 512:(h + 1) * 512],
                                start=(t == 0), stop=(t == NBLK - 1),
                                skip_group_check=True)

                    DELAY = 3
                    pending = []
                    for t in range(NBLK):
                        pt = psum.tile([128, GRP], f32, tag="psum",
                                       name=f"psB{t}_{sg}{sfx}")
                        for u in range(GRP // 512):
                            c0 = sg * GRP + u * 512
                            nc.tensor.matmul(
                                out=pt[:, u * 512:(u + 1) * 512],
                                lhsT=lu[:, t * 128:(t + 1) * 128],
                                rhs=rq[:, c0:c0 + 512],
                                start=True, stop=True)
                        if len(pending) >= DELAY:
                            colsums(*pending.pop(0))
                        mask = maskp.tile([128, GRP], bf16, tag="mask",
                                          name=f"mk{t}_{sg}{sfx}")
                        epilogue(pt,
                                 parts[:, t * 6 + NG_A + sg:
                                       t * 6 + NG_A + sg + 1],
                                 next(it_assign), mask=mask)
                        pending.append((t, mask))
                    for p in pending:
                        colsums(*p)
                    for h in range(2):
                        nc.vector.tensor_copy(
                            colsum_sb[0:1,
                                      sg * GRP + h * 512: sg * GRP + (h + 1) * 512],
                            pc[h][0:1, :])

                # batched reduces: parts viewed [128, 32 blocks, 6 cols]
                parts3 = parts.rearrange("p (t c) -> p t c", c=NG_A + NG_B)
                nc.vector.reduce_sum(cdiag, parts3[:, :, 0:NG_A],
                                     axis=mybir.AxisListType.X)
                nc.vector.reduce_sum(cq12, parts3[:, :, NG_A:NG_A + NG_B],
                                     axis=mybir.AxisListType.X)

                nc.sync.dma_start(out=cdiag_d[:, :], in_=cdiag)
                nc.sync.dma_start(out=cq12_d[:, :], in_=cq12)
                nc.sync.dma_start(out=colsum_d[:, :], in_=colsum_sb)

            for rep in range(repeat):
                body(rep)

    split_excess_waits(nc)
    return nc


def _build_v4(repeat=1):
    """v4: PE array tiling + long fused epilogue groups.

    Phase A (diag quadrant, rowsums only): 4x row tiling (32x128 mode).
    The staircase rhs layout puts column chunk 512*i of each 2048-col group
    on SBUF partition strip 32*i, so the 4 concurrently-running PE tiles
    fill one [128, 2048] 4-bank PSUM group that all belongs to ONE row
    tile -> a single V/S pass with accum_out drains 2048 columns at once.

    Phase B (q12 strip, rowsums + masks -> PE colsums): 2x row tiling
    (64x128 mode), groups of [128, 1024]; colsum accumulation identical to
    v3 (ones-stationary matmuls into persistent PSUM accumulators).
    """
    bf16 = mybir.dt.bfloat16
    f32 = mybir.dt.float32
    SIG = mybir.ActivationFunctionType.Sigmoid

    nc = bass.Bass()
    ls_d = nc.dram_tensor("lhs_self", [KAUG, HALF], bf16, kind="ExternalInput")
    lu_d = nc.dram_tensor("lhs_up", [KAUG, HALF], bf16, kind="ExternalInput")
    rd_d = nc.dram_tensor("rhs_diag", [KAUG, HALF], bf16, kind="ExternalInput")
    rq_d = nc.dram_tensor("rhs_q12", [KAUG, QCOL], bf16, kind="ExternalInput")
    cdiag_d = nc.dram_tensor("cdiag", [128, NBLK], f32, kind="ExternalOutput")
    cq12_d = nc.dram_tensor("cq12", [128, NBLK], f32, kind="ExternalOutput")
    colsum_d = nc.dram_tensor("colsum", [1, QCOL], f32, kind="ExternalOutput")

    NGA = 2                  # [128, 2048] groups per row tile, phase A
    NGB = 2                  # [128, 1024] groups per row tile, phase B
    SLOTS = NGA + NGB
    S_SHARE = 0.55           # ScalarE fraction of epilogue groups

    def mk_assign(n):
        out = []
        acc = 0.0
        for _ in range(n):
            acc += S_SHARE
            if acc >= 1.0:
                out.append("A")
                acc -= 1.0
            else:
                out.append("D")
        return out

    with tile.TileContext(nc) as tc:
        with (
            tc.tile_pool(name="inp", bufs=1) as inp,
            tc.tile_pool(name="masks", bufs=6) as maskp,
            tc.tile_pool(name="fin", bufs=1) as fin,
        ):
            ls4 = inp.tile([128, HALF], bf16, name="ls4")   # 4-strip replica
            lu2 = inp.tile([128, HALF], bf16, name="lu2")   # 2-strip replica
            rdS = inp.tile([128, 1024], bf16, name="rdS")   # diag staircase
            rqS = inp.tile([128, 1024], bf16, name="rqS")   # q12 staircase
            ones = fin.tile([128, 128], bf16, name="ones")
            # zero all padding rows once (K=32/64 APs read them).
            nc.gpsimd.memset(ls4[:, :].bitcast(mybir.dt.uint32), 0)
            nc.gpsimd.memset(lu2[:, :].bitcast(mybir.dt.uint32), 0)
            nc.scalar.memzero(rdS[:, :])
            nc.gpsimd.memset(rqS[:, :].bitcast(mybir.dt.uint32), 0)
            nc.gpsimd.memset(ones, 1.0)

            parts = fin.tile([128, NBLK * SLOTS], f32, name="parts")
            cdiag = fin.tile([128, NBLK], f32, name="cdiag")
            cq12 = fin.tile([128, NBLK], f32, name="cq12")
            colsum_sb = fin.tile([1, QCOL], f32, name="colsum")

            rdv = rd_d[:, :].rearrange("k (g i c) -> k g i c", g=2, i=4, c=512)
            rqv = rq_d[:, :].rearrange("k (g j c) -> k g j c", g=2, j=2, c=512)

            def load_inputs(rep):
                # lhs replicas: same DRAM source to each strip.
                for i in range(4):
                    nc.sync.dma_start(
                        out=ls4[32 * i:32 * i + KAUG, :], in_=ls_d[:, :])
                for j in range(2):
                    nc.sync.dma_start(
                        out=lu2[64 * j:64 * j + KAUG, :], in_=lu_d[:, :])
                # staircases: strip s holds cols {512*s + G*g + c}.
                for i in range(4):
                    dst = rdS[32 * i:32 * i + KAUG, :].rearrange(
                        "k (g c) -> k g c", g=2)
                    nc.sync.dma_start(out=dst, in_=rdv[:, :, i, :])
                for j in range(2):
                    dst = rqS[64 * j:64 * j + KAUG, :].rearrange(
                        "k (g c) -> k g c", g=2)
                    nc.sync.dma_start(out=dst, in_=rqv[:, :, j, :])

            def epilogue(pt, slot, eng, mask=None):
                out_t = mask if mask is not None else pt
                if eng == "A":
                    nc.scalar.activation(out_t, pt, SIG, bias=0.0,
                                         scale=1.0e6, accum_out=slot)
                else:
                    nc.vector.tensor_scalar(out_t, pt, 0.0, None,
                                            mybir.AluOpType.is_gt,
                                            mybir.AluOpType.add,
                                            accum_out=slot)

            def body(rep):
                load_inputs(rep)

                # ---- phase A: diag quadrant, 4x tiling ----
                itA = iter(mk_assign(NBLK * NGA))
                with tc.tile_pool(name="psA", bufs=2, space="PSUM") as psA:
                    for t in range(NBLK):
                        for g in range(NGA):
                            pt = psA.tile([128, 2048], f32, tag="psA",
                                          name=f"psA{t}_{g}_r{rep}")
                            for i in range(4):
                                nc.tensor.matmul(
                                    out=pt[:, 512 * i:512 * (i + 1)],
                                    lhsT=ls4[32 * i:32 * (i + 1),
                                             t * 128:(t + 1) * 128],
                                    rhs=rdS[32 * i:32 * (i + 1),
                                            g * 512:(g + 1) * 512],
                                    start=True, stop=True,
                                    tile_position=(32 * i, 0))
                            epilogue(pt,
                                     parts[:, t * SLOTS + g:t * SLOTS + g + 1],
                                     next(itA))

                # ---- phase B: q12 strip, 2x tiling + colsums ----
                itB = iter(mk_assign(NBLK * NGB))
                with (
                    tc.tile_pool(name="psB", bufs=2, space="PSUM") as psB,
                    tc.tile_pool(name="pscol", bufs=2, space="PSUM") as pscol,
                ):
                    for g in range(NGB):
                        pc = [pscol.tile([128, 512], f32, tag="pscol",
                                         name=f"pc{g}_{h}_r{rep}")
                              for h in range(2)]

                        def colsums(t, mask):
                            for h in range(2):
                                nc.tensor.matmul(
                                    out=pc[h][:, :],
                                    lhsT=ones[:, :],
                                    rhs=mask[:, h * 512:(h + 1) * 512],
                                    start=(t == 0), stop=(t == NBLK - 1),
                                    skip_group_check=True)

                        DELAY = 3
                        pending = []
                        for t in range(NBLK):
                            pt = psB.tile([128, 1024], f32, tag="psB",
                                          name=f"psB{t}_{g}_r{rep}")
                            for j in range(2):
                                nc.tensor.matmul(
                                    out=pt[:, 512 * j:512 * (j + 1)],
                                    lhsT=lu2[64 * j:64 * j + 64,
                                             t * 128:(t + 1) * 128],
                                    rhs=rqS[64 * j:64 * j + 64,
                                            g * 512:(g + 1) * 512],
                                    start=True, stop=True,
                                    tile_position=(64 * j, 0))
                            if len(pending) >= DELAY:
                                colsums(*pending.pop(0))
                            mask = maskp.tile([128, 1024], bf16, tag="mask",
                                              name=f"mk{t}_{g}_r{rep}")
                            epilogue(pt,
                                     parts[:, t * SLOTS + NGA + g:
                                           t * SLOTS + NGA + g + 1],
                                     next(itB), mask=mask)
                            pending.append((t, mask))
                        for p in pending:
                            colsums(*p)
                        for h in range(2):
                            nc.vector.tensor_copy(
                                colsum_sb[0:1, g * 1024 + h * 512:
                                          g * 1024 + (h + 1) * 512],
                                pc[h][0:1, :])

                parts4 = parts.rearrange("p (t c) -> p t c", c=SLOTS)
                nc.vector.reduce_sum(cdiag, parts4[:, :, 0:NGA],
                                     axis=mybir.AxisListType.X)
                nc.vector.reduce_sum(cq12, parts4[:, :, NGA:SLOTS],
                                     axis=mybir.AxisListType.X)

                nc.sync.dma_start(out=cdiag_d[:, :], in_=cdiag)
                nc.sync.dma_start(out=cq12_d[:, :], in_=cq12)
                nc.sync.dma_start(out=colsum_d[:, :], in_=colsum_sb)

            for rep in range(repeat):
                body(rep)

    split_excess_waits(nc)
    return nc


BUILDER = _build_v4


# ---------------------------------------------------------------------------
# Cached AOT dispatch (compile once, reuse the PJRT executable every call).
# ---------------------------------------------------------------------------
class CompiledBass:
    def __init__(self, nc, n_cores):
        install_neuronx_cc_hook()
        assert nc.dbg_addr is None
        partition_name = (
            nc.partition_id_tensor.name if nc.partition_id_tensor else None)
        in_names, out_names, out_avals = [], [], []
        in_shapes, in_dtypes = [], []
        for alloc in nc.m.functions[0].allocations:
            if not isinstance(alloc, mybir.MemoryLocationSet):
                continue
            name = alloc.memorylocations[0].name
            if alloc.kind == "ExternalInput":
                if name != partition_name:
                    in_names.append(name)
                    in_shapes.append(tuple(alloc.tensor_shape))
                    in_dtypes.append(mybir.dt.np(alloc.dtype))
            elif alloc.kind == "ExternalOutput":
                out_names.append(name)
                out_avals.append(jax.core.ShapedArray(
                    tuple(alloc.tensor_shape), mybir.dt.np(alloc.dtype)))
        self.n_cores = n_cores
        self.in_names = in_names
        self.out_names = out_names
        self.out_shapes = [tuple(a.shape) for a in out_avals]
        self.out_dtypes = [a.dtype for a in out_avals]
        all_in_names = list(in_names)
        if partition_name is not None:
            all_in_names.append(partition_name)

        def _body(*args):
            operands = list(args)
            if partition_name is not None:
                operands.append(partition_id_tensor())
            return tuple(_bass_exec_p.bind(
                *operands, out_avals=tuple(out_avals),
                in_names=tuple(all_in_names),
                out_names=tuple(out_names), lowering_input_output_aliases=(),
                sim_require_finite=True, sim_require_nnan=True, nc=nc))

        devices = jax.devices()[:n_cores]
        assert len(devices) == n_cores, (len(devices), n_cores)
        self.mesh = Mesh(np.asarray(devices), ("core",))
        in_specs = (PartitionSpec("core"),) * len(in_names)
        out_specs = (PartitionSpec("core"),) * len(out_names)
        arg_shapes = [
            jax.ShapeDtypeStruct((n_cores * s[0], *s[1:]), d)
            for s, d in zip(in_shapes, in_dtypes)
        ]

        def compile_fn():
            return jax.jit(
                shard_map(_body, mesh=self.mesh, in_specs=in_specs,
                          out_specs=out_specs, check_rep=False),
                keep_unused=True,
            ).lower(*arg_shapes).compile()

        self.compiled = fast_dispatch_compile(compile_fn)

    def __call__(self, concat_inputs):
        """concat_inputs: np/jax arrays concatenated on axis 0 across cores,
        in self.in_names order. Returns list of per-core output dicts."""
        outs = self.compiled(*concat_inputs)
        res = []
        for c in range(self.n_cores):
            d = {}
            for i, name in enumerate(self.out_names):
                s = self.out_shapes[i]
                d[name] = np.asarray(outs[i]).reshape(self.n_cores, *s)[c]
            res.append(d)
        return res


def get_compiled(repeat=1):
    key = ("cb", repeat)
    if key not in _cache:
        _cache[key] = CompiledBass(BUILDER(repeat=repeat), N_CORES)
    return _cache[key]


def _prep_inputs(point_features):
    """Per-core input blocks, concatenated core-major on axis 0 (the layout
    CompiledBass expects). Returns dict name -> [8*KAUG, cols] bf16."""
    x = np.asarray(point_features, dtype=np.float32)
    xb = x.astype(ml_dtypes.bfloat16)
    xf = xb.astype(np.float32)                      # bf16-rounded features
    sq = np.einsum("bnd,bnd->bn", xf, xf)           # [B, N] f32
    a = sq / 2.0
    nb = (0.25 - sq) / 2.0                          # -b_j

    ls = np.empty((N_CORES, KAUG, HALF), np.float32)
    lu = np.empty((N_CORES, KAUG, HALF), np.float32)
    rd = np.empty((N_CORES, KAUG, HALF), np.float32)
    rq = np.empty((N_CORES, KAUG, QCOL), np.float32)
    for c in range(N_CORES):
        b, r = c // 2, c % 2
        self_rows = slice(r * HALF, (r + 1) * HALF)
        q12_cols = slice(HALF + r * QCOL, HALF + (r + 1) * QCOL)
        for dst, rows, kind in (
            (ls[c], self_rows, "lhs"), (lu[c], slice(0, HALF), "lhs"),
            (rd[c], self_rows, "rhs"), (rq[c], q12_cols, "rhs"),
        ):
            dst[0:D] = xf[b, rows].T
            if kind == "lhs":
                dst[D] = 1.0
                dst[D + 1] = a[b, rows]
            else:
                dst[D] = nb[b, rows]
                dst[D + 1] = -1.0
    cast = ml_dtypes.bfloat16
    return {
        "lhs_self": ls.reshape(N_CORES * KAUG, HALF).astype(cast),
        "lhs_up": lu.reshape(N_CORES * KAUG, HALF).astype(cast),
        "rhs_diag": rd.reshape(N_CORES * KAUG, HALF).astype(cast),
        "rhs_q12": rq.reshape(N_CORES * KAUG, QCOL).astype(cast),
    }


def _merge_outputs(res):
    out = np.empty((B, N), dtype=np.int32)
    for b in range(B):
        A, Bc = res[2 * b], res[2 * b + 1]
        up = (A["cdiag"].T.reshape(HALF) + A["cq12"].T.reshape(HALF)
              + Bc["cq12"].T.reshape(HALF))
        lo = (Bc["cdiag"].T.reshape(HALF)
              + np.concatenate([A["colsum"][0], Bc["colsum"][0]]))
        counts = np.concatenate([up, lo])
        out[b] = np.where(counts < 9.5, -1, 0).astype(np.int32)
    return out


def kernel(point_features):
    cb = get_compiled()
    inp = _prep_inputs(point_features)
    res = cb([inp[nm] for nm in cb.in_names])
    return _merge_outputs(res)


if __name__ == "__main__":
    x = np.random.default_rng(0).standard_normal((B, N, D)).astype(np.float32)
    y = kernel(x)
    print("out shape/dtype:", y.shape, y.dtype, "uniq:", np.unique(y))
